# revision 1
# baseline (speedup 1.0000x reference)
"""Trainium2 Bass kernel for NeighborhoodNormalization.

Math: the reference builds a per-point homogeneous transform
T = [[ux,-uy,0,px],[uy,ux,0,py],[0,0,1,pz],[0,0,0,1]] (u = p/||p||),
inverts it, and applies it to 64 neighbors per point.  The inverse has a
closed form: with r2 = px^2+py^2, n = ||p||, a = n/r2, cx = px*a, cy = py*a:

    out.x =  cx*qx + cy*qy + tx      tx = -(cx*px + cy*py)
    out.y = -cy*qx + cx*qy + ty      ty =  (cy*px - cx*py)
    out.z =  qz - pz

So the kernel is pure elementwise math (memory-bound).  Sharding: pure data
parallel over the N=8192 point axis across 8 cores (1024 points/core).

Per-core layout: 16384 points = 128 partitions x 128 columns, where
partition p = b*8 + s holds points with local n = s*128 + t (t = column).
Neighbor rows (64*3 floats) stay contiguous in HBM per point, so DMAs are
[128 partitions x G*768B contiguous] blocks.  Per-point coefficients live as
[128,128] SBUF tiles; column t supplies the per-partition scalars for the
fused tensor_scalar / scalar_tensor_tensor / activation ops of column t.
"""

import sys

if "/opt/trn_rl_repo" not in sys.path:
    sys.path.insert(0, "/opt/trn_rl_repo")

import numpy as np

import concourse.bass as bass
import concourse.bacc as bacc
import concourse.mybir as mybir
from concourse.tile import TileContext
from concourse.bass_utils import run_bass_kernel_spmd

B = 16
N = 8192
K = 64
NCORES = 8
NLOC = N // NCORES  # 1024 points per core
P = 128             # SBUF partitions
S = NLOC // P       # 8 partition sub-blocks per batch entry
T = (B * NLOC) // P  # 128 point-columns per partition
G = 16              # columns per DMA group
NG = T // G

F32 = mybir.dt.float32
OP = mybir.AluOpType
AF = mybir.ActivationFunctionType

_CACHE = {}


def _build_nc():
    nc = bacc.Bacc(None, target_bir_lowering=False)

    pts = nc.declare_dram_parameter("points", [B, NLOC, 3], F32, isOutput=False)
    nb = nc.declare_dram_parameter("neighborhoods", [B, NLOC, K, 3], F32, isOutput=False)
    out = nc.declare_dram_parameter("out", [B, NLOC, K, 3], F32, isOutput=True)

    # partition = (b s), columns = t, free = 192 floats per point
    nbr = nb[:].rearrange("b (s t) k c -> (b s) t (k c)", s=S)
    outr = out[:].rearrange("b (s t) k c -> (b s) t (k c)", s=S)
    ptsr = pts[:].rearrange("b (s t) c -> (b s) (t c)", s=S)

    with TileContext(nc) as tc:
        with tc.tile_pool(name="const", bufs=1) as cpool, \
             tc.tile_pool(name="io_in", bufs=6) as inpool, \
             tc.tile_pool(name="io_out", bufs=6) as outpool, \
             tc.tile_pool(name="tmp", bufs=16) as tmppool:

            pts_sb = cpool.tile([P, T * 3], F32, tag="pts")
            nc.sync.dma_start(out=pts_sb[:], in_=ptsr)
            pv = pts_sb[:].rearrange("p (t c) -> p t c", c=3)
            px = pv[:, :, 0]
            py = pv[:, :, 1]
            pz = pv[:, :, 2]

            def ctile(tag):
                return cpool.tile([P, T], F32, tag=tag, name=tag)

            t1 = ctile("t1")
            t2 = ctile("t2")
            r2 = ctile("r2")
            n2 = ctile("n2")
            nn = ctile("nn")
            ir2 = ctile("ir2")
            aa = ctile("aa")
            cx = ctile("cx")
            cy = ctile("cy")
            ncy = ctile("ncy")
            tx = ctile("tx")
            ty = ctile("ty")
            npz = ctile("npz")

            nc.vector.tensor_mul(out=t1[:], in0=px, in1=px)
            nc.vector.tensor_mul(out=t2[:], in0=py, in1=py)
            nc.vector.tensor_add(out=r2[:], in0=t1[:], in1=t2[:])
            nc.vector.tensor_mul(out=t1[:], in0=pz, in1=pz)
            nc.vector.tensor_add(out=n2[:], in0=r2[:], in1=t1[:])
            nc.scalar.sqrt(out=nn[:], in_=n2[:])
            nc.vector.reciprocal(out=ir2[:], in_=r2[:])
            nc.vector.tensor_mul(out=aa[:], in0=nn[:], in1=ir2[:])
            nc.vector.tensor_mul(out=cx[:], in0=px, in1=aa[:])
            nc.vector.tensor_mul(out=cy[:], in0=py, in1=aa[:])
            nc.vector.tensor_scalar_mul(out=ncy[:], in0=cy[:], scalar1=-1.0)
            # tx = -(cx*px + cy*py)
            nc.vector.tensor_mul(out=t1[:], in0=cx[:], in1=px)
            nc.vector.tensor_mul(out=t2[:], in0=cy[:], in1=py)
            nc.vector.tensor_add(out=t1[:], in0=t1[:], in1=t2[:])
            nc.vector.tensor_scalar_mul(out=tx[:], in0=t1[:], scalar1=-1.0)
            # ty = cy*px - cx*py
            nc.vector.tensor_mul(out=t1[:], in0=cy[:], in1=px)
            nc.vector.tensor_mul(out=t2[:], in0=cx[:], in1=py)
            nc.vector.tensor_sub(out=ty[:], in0=t1[:], in1=t2[:])
            nc.vector.tensor_scalar_mul(out=npz[:], in0=pz, scalar1=-1.0)

            for g in range(NG):
                nb_t = inpool.tile([P, G, K, 3], F32, tag="nb", name=f"nb{g}")
                nc.sync.dma_start(
                    out=nb_t[:].rearrange("p g k c -> p g (k c)"),
                    in_=nbr[:, g * G:(g + 1) * G, :],
                )
                ot = outpool.tile([P, G, K, 3], F32, tag="ot", name=f"ot{g}")
                # out.z = qz - pz for the whole group in one wide op
                # (npz broadcast along the K axis via 0-stride AP)
                npz_b = npz[:, g * G:(g + 1) * G, None].broadcast_to([P, G, K])
                nc.vector.tensor_add(
                    out=ot[:, :, :, 2], in0=nb_t[:, :, :, 2], in1=npz_b,
                )
                for i in range(G):
                    t = g * G + i
                    qx = nb_t[:, i, :, 0]
                    qy = nb_t[:, i, :, 1]
                    ox = ot[:, i, :, 0]
                    oy = ot[:, i, :, 1]
                    cx_t = cx[:, t:t + 1]
                    cy_t = cy[:, t:t + 1]
                    ncy_t = ncy[:, t:t + 1]
                    tx_t = tx[:, t:t + 1]
                    ty_t = ty[:, t:t + 1]

                    # i2 = cy*qy + tx   (ACT: Identity(in*scale + bias))
                    tmp1 = tmppool.tile([P, K], F32, tag="tmp1", name=f"tmp1_{t}")
                    nc.scalar.activation(
                        out=tmp1[:], in_=qy, func=AF.Identity,
                        bias=tx_t, scale=cy_t,
                    )
                    # out.x = cx*qx + i2
                    nc.vector.scalar_tensor_tensor(
                        out=ox, in0=qx, scalar=cx_t, in1=tmp1[:],
                        op0=OP.mult, op1=OP.add,
                    )
                    # j2 = -cy*qx + ty   (GpSimd: otherwise idle)
                    tmp2 = tmppool.tile([P, K], F32, tag="tmp2", name=f"tmp2_{t}")
                    nc.gpsimd.tensor_scalar(
                        out=tmp2[:], in0=qx, scalar1=ncy_t, scalar2=ty_t,
                        op0=OP.mult, op1=OP.add,
                    )
                    # out.y = cx*qy + j2
                    nc.vector.scalar_tensor_tensor(
                        out=oy, in0=qy, scalar=cx_t, in1=tmp2[:],
                        op0=OP.mult, op1=OP.add,
                    )
                # out-DMA on the ACT HWDGE ring so it overlaps the SP-ring
                # input stream (HWDGE is FIFO per issuing engine).
                nc.scalar.dma_start(
                    out=outr[:, g * G:(g + 1) * G, :],
                    in_=ot[:].rearrange("p g k c -> p g (k c)"),
                )

    nc.compile()
    return nc


def _get_nc():
    if "nc" not in _CACHE:
        _CACHE["nc"] = _build_nc()
    return _CACHE["nc"]


def kernel(points, neighborhoods):
    pts = np.ascontiguousarray(np.asarray(points, dtype=np.float32))
    nb = np.ascontiguousarray(np.asarray(neighborhoods, dtype=np.float32))
    assert pts.shape == (B, N, 3), pts.shape
    assert nb.shape == (B, N, K, 3), nb.shape

    in_maps = []
    for c in range(NCORES):
        sl = slice(c * NLOC, (c + 1) * NLOC)
        in_maps.append({
            "points": np.ascontiguousarray(pts[:, sl]),
            "neighborhoods": np.ascontiguousarray(nb[:, sl]),
        })

    res = run_bass_kernel_spmd(_get_nc(), in_maps, list(range(NCORES))).results
    out = np.concatenate([res[c]["out"] for c in range(NCORES)], axis=1)
    return out

